# revision 1
# baseline (speedup 1.0000x reference)
"""Trainium2 Bass kernel for nn_BSplineScheduler.

Evaluates a clamped cubic B-spline (32 coeffs from theta, fixed uniform
knots, 31 active spans) at M=4194304 points, data-parallel over 8
NeuronCores.

Algorithm: on [k/31, (k+1)/31) the spline is a cubic in t = 31x - k.
Writing P_k(t) = v_k + T_k(t) with T_k(0) = 0, C0-continuity gives the
exact telescoping form

    S(x) = sum_{k=0}^{30} T_k( clamp(31x - k, 0, 1) )

(terms with k < span saturate to T_k(1) and telescope to v_span; terms
with k > span are exactly 0).  Per knot the device does one ScalarE
activation u' = relu(31x - k) and one fused custom-DVE op
acc += T_k(min(u', 1)) (7 ALU stages).  The T_k coefficients are
computed host-side in float64 from theta and baked into the
instructions as compile-time immediates, so the device program has a
single streamed input (the points) and a single output.
"""

import numpy as np

_M = 4194304
_NCORES = 8
_P = 128
_FD = 4096          # per-core free dim: 8 * 128 * 4096 = 4194304
_FD_TILE = 2048
_NKNOTS = 31

_N_COEFF = 32
_ORDER = 4
_N_TOTAL = _N_COEFF + 2

_cache = {}

TRACE = False
LAST_RESULTS = None


# --------------------------------------------------------------------------
# Host-side math: theta -> per-span cubic coefficients (float64)
# --------------------------------------------------------------------------

def _knots():
    interior = np.linspace(0.0, 1.0, _N_TOTAL - _ORDER + 2)
    return np.concatenate([np.zeros(_ORDER - 1), interior, np.ones(_ORDER - 1)])


def _coefficients(theta):
    t = np.asarray(theta, dtype=np.float64)
    deltas = np.log1p(np.exp(-np.abs(t))) + np.maximum(t, 0.0)   # softplus
    cs = np.cumsum(deltas)
    return np.concatenate([[0.0], cs / cs[-1], [1.0]])           # [34]


def _basis_matrix(sc, kn):
    n_spans = len(kn) - 1
    left, right = kn[:-1], kn[1:]
    b = ((sc[:, None] >= left) & (sc[:, None] < right)).astype(np.float64)
    b[:, -1] = ((sc >= left[-1]) & (sc <= right[-1])).astype(np.float64)
    for p in range(2, _ORDER + 1):
        m = n_spans - p + 1
        i = np.arange(m)
        d1 = kn[i + p - 1] - kn[i]
        d2 = kn[i + p] - kn[i + 1]
        s1 = np.abs(d1) > 1e-10
        s2 = np.abs(d2) > 1e-10
        w1 = np.where(s1, (sc[:, None] - kn[i]) / np.where(s1, d1, 1.0), 0.0)
        w2 = np.where(s2, (kn[i + p] - sc[:, None]) / np.where(s2, d2, 1.0), 0.0)
        b = w1 * b[:, :m] + w2 * b[:, 1 : m + 1]
    return b[:, :_N_TOTAL]


def _span_table(theta):
    """[31, 4] coefficients of S restricted to span k, in t = 31x - k."""
    kn = _knots()
    c = _coefficients(theta)
    tn = np.array([0.125, 0.375, 0.625, 0.875])
    V = np.vander(tn, 4, increasing=True)
    R = np.zeros((_NKNOTS, 4))
    for k in range(_NKNOTS):
        xs = (k + tn) / 31.0
        vals = _basis_matrix(xs, kn) @ c
        R[k] = np.linalg.solve(V, vals)
    return R


# --------------------------------------------------------------------------
# Custom DVE ops
# --------------------------------------------------------------------------

def _register_ops():
    import concourse.dve_ops as dve_ops
    from concourse.dve_spec import Spec, Src0, Src1, C0, C1, C2, One, minn, lower
    from concourse.dve_uop import DveOpSpec

    def reg(name, spec, rd1_en):
        for op in dve_ops.OPS:
            if op.name == name:
                return op
        opcode = dve_ops._CUSTOM_DVE_ROW_BASE + len(dve_ops.OPS)
        assert opcode < 0x20
        shas = {}
        for ver in ("v3", "v4"):
            uops = lower(spec, ver=ver)
            shas[ver] = DveOpSpec(
                name=name, opcode=opcode, uops=uops, rd1_en=rd1_en
            ).sha(ver)
        op = dve_ops.DveOp(name, spec, False, shas)
        dve_ops.OPS.append(op)
        dve_ops.CUSTOM_DVE_SPECS[name] = spec
        dve_ops._SUB_OPCODE_FOR_NAME[name] = opcode
        return op

    ua = minn(Src0, One)
    acc_op = reg(
        "BSPL_ACC", Spec(body=((ua * C2 + C1) * ua + C0) * ua + Src1), rd1_en=True
    )
    us = minn(Src0, One)
    seed_op = reg(
        "BSPL_SEED", Spec(body=((us * C2 + C1) * us + C0) * us), rd1_en=False
    )
    return seed_op, acc_op


# --------------------------------------------------------------------------
# Device program
# --------------------------------------------------------------------------

def _build_and_compile(R):
    import concourse.bacc as bacc
    import concourse.mybir as mybir
    import concourse.tile as tile
    import concourse.bass as bass

    seed_op, acc_op = _register_ops()

    r1 = [float(R[k, 1]) for k in range(_NKNOTS)]
    r2 = [float(R[k, 2]) for k in range(_NKNOTS)]
    r3 = [float(R[k, 3]) for k in range(_NKNOTS)]

    nc = bacc.Bacc("TRN2", target_bir_lowering=False, debug=False)

    # const [128,1] APs for the per-knot activation biases -k
    for k in range(_NKNOTS):
        val = float(-k)
        if (mybir.dt.float32, val) not in nc.const_aps.aps:
            t = nc.alloc_sbuf_tensor(f"const-bias-{k}", [128, 1], mybir.dt.float32)
            nc.gpsimd.memset(t.ap(), val)
            nc.const_aps.aps[(mybir.dt.float32, val)] = t.ap()
    nc.all_engine_barrier()

    x_in = nc.declare_dram_parameter("s", [_P, _FD], mybir.dt.float32, isOutput=False)
    out = nc.declare_dram_parameter("out", [_P, _FD], mybir.dt.float32, isOutput=True)

    with tile.TileContext(nc) as tc:
        with (
            tc.tile_pool(name="xs", bufs=2) as xpool,
            tc.tile_pool(name="ups", bufs=4) as upool,
            tc.tile_pool(name="accs", bufs=3) as apool,
        ):
            for ti in range(_FD // _FD_TILE):
                xt = xpool.tile([_P, _FD_TILE], mybir.dt.float32, tag="x")
                nc.sync.dma_start(xt[:], x_in[:, bass.ts(ti, _FD_TILE)])
                acc = None
                for k in range(_NKNOTS):
                    up = upool.tile([_P, _FD_TILE], mybir.dt.float32, tag="up")
                    nc.scalar.activation(
                        up[:], xt[:], mybir.ActivationFunctionType.Relu,
                        bias=float(-k), scale=31.0,
                    )
                    newacc = apool.tile([_P, _FD_TILE], mybir.dt.float32, tag="acc")
                    if acc is None:
                        nc.vector._custom_dve(
                            seed_op, out=newacc[:], in0=up[:],
                            s0=r1[k], s1=r2[k], imm2=r3[k],
                        )
                    else:
                        nc.vector._custom_dve(
                            acc_op, out=newacc[:], in0=up[:], in1=acc[:],
                            s0=r1[k], s1=r2[k], imm2=r3[k],
                        )
                    acc = newacc
                nc.sync.dma_start(out[:, bass.ts(ti, _FD_TILE)], acc[:])

    nc.compile()
    return nc


# --------------------------------------------------------------------------
# Entry point
# --------------------------------------------------------------------------

def kernel(s, theta):
    global LAST_RESULTS
    from concourse.bass_utils import run_bass_kernel_spmd

    s = np.asarray(s)
    orig_shape = s.shape
    R = _span_table(np.asarray(theta))

    key = R.tobytes()
    if key not in _cache:
        _cache[key] = _build_and_compile(R)
    nc = _cache[key]

    xs = np.ascontiguousarray(s.astype(np.float32).reshape(_NCORES, _P, _FD))
    in_maps = [{"s": xs[c]} for c in range(_NCORES)]
    res = run_bass_kernel_spmd(
        nc, in_maps, core_ids=list(range(_NCORES)), trace=TRACE
    )
    LAST_RESULTS = res
    out = np.concatenate(
        [np.asarray(res.results[c]["out"]).reshape(-1) for c in range(_NCORES)]
    )
    return out.reshape(orig_shape).astype(np.float32)
